# revision 24
# baseline (speedup 1.0000x reference)
"""Trainium2 Bass kernel for a per-channel-pair 2x2 unitary mixing layer.

Math (reference): for each channel pair g (C=2048 -> G=1024 pairs):
    M[g] = R(phase[g]) @ polar_project(W_pairs[g])        # 2x2
    y[..., 2g:2g+2] = M[g] @ x[..., 2g:2g+2]

Device formulation (token-major):
    A[2g] = M[g,0,0]; A[2g+1] = M[g,1,1]
    B[2g] = M[g,0,1]; B[2g+1] = M[g,1,0]
    y[c] = A[c] * x[c] + B[c] * x[partner(c)]
The tiny 2x2 SVD/compose prep runs on host (same formula as the
reference); the [B,T,C] transform runs on 8 NeuronCores, data parallel
over batch (x[b] per core), with coefficient tensors replicated.

Per core the work is split across engine pipelines so the kernel rides
the HBM roofline (~206 us for 64 MiB at the per-NC limit):
  - DVE path (6 of 8 slabs): three fp32 tensor_tensor ops per stripe
    (t = x*A; u = swap(x)*B; y = t + u). ~27.5 us/slab.
  - PE path (2 of 8 slabs): per 128x128 block, PE transpose to PSUM,
    ACT copy to SBUF, PE matmul against the host-built block-diagonal
    W chunk (y = x @ Wblk), ACT copy back. ~45 us/slab, runs on
    otherwise-idle engines.
GPSIMD is left idle: its SBUF port serializes against DVE 2-read ops
(exclusive shared-port lock, measured). All DMAs are HWDGE (nc.sync),
4 MiB per transfer.
"""

import sys

if "/opt/trn_rl_repo" not in sys.path:
    sys.path.insert(0, "/opt/trn_rl_repo")

import numpy as np

# Problem shape (hardcoded per harness contract)
B_FULL, T_FULL, C = 8, 4096, 2048
G = C // 2
N_CORES = 8
TOK = T_FULL  # tokens per core after batch sharding: 4096
P = 128  # SBUF partitions
S = 4  # token-stripes per slab
SS = TOK // (S * P)  # slabs per core: 8
NBLK = C // P  # 128-channel blocks per stripe: 16

# Global stripes processed on the PE (transpose+matmul) path; the rest go
# to the DVE path. ~1 of 3 balances PE (~12 us/stripe wall) against DVE
# (~6.9 us/stripe) under the ~162 us DMA floor.
def _is_pe_stripe(k):
    return k % 3 == 2

_NC_CACHE = {}


def _prep_coeffs(W_pairs: np.ndarray, phase: np.ndarray):
    """Host prep: M = R(phase) @ polar(W).

    Returns per-channel coefficient vectors A, B and the block-diagonal
    weight chunks W_dram [P, C] used by the PE path (W_dram[p, j*128+n]
    = Wblk_j[p, n], y_block = x_block @ Wblk_j).
    """
    W = np.asarray(W_pairs, dtype=np.float32)
    ph = np.asarray(phase, dtype=np.float32)
    # Same math as the reference: polar factor via SVD (U @ Vh), fp32.
    U, _, Vh = np.linalg.svd(W)
    Q = U @ Vh  # [G,2,2]
    c, s = np.cos(ph), np.sin(ph)
    R = np.stack([np.stack([c, -s], -1), np.stack([s, c], -1)], -2)  # [G,2,2]
    M = np.einsum("gij,gjk->gik", R, Q).astype(np.float32)  # [G,2,2]

    A = np.empty(C, dtype=np.float32)
    Bc = np.empty(C, dtype=np.float32)
    A[0::2] = M[:, 0, 0]
    A[1::2] = M[:, 1, 1]
    Bc[0::2] = M[:, 0, 1]
    Bc[1::2] = M[:, 1, 0]

    # Block-diagonal chunks: Wblk_j[2gl+jj, 2gl+ii] = M[j*64+gl, ii, jj]
    Wblk = np.zeros((NBLK, P, P), dtype=np.float32)
    gl = np.arange(P // 2)
    for j in range(NBLK):
        Mj = M[j * (P // 2) + gl]  # [64,2,2]
        for jj in range(2):
            for ii in range(2):
                Wblk[j, 2 * gl + jj, 2 * gl + ii] = Mj[:, ii, jj]
    W_dram = np.ascontiguousarray(Wblk.transpose(1, 0, 2).reshape(P, C))
    return A, Bc, W_dram


def _build_nc():
    """Build the single-core Bass program (SPMD across 8 cores)."""
    if "nc" in _NC_CACHE:
        return _NC_CACHE["nc"]

    import concourse.bacc as bacc
    import concourse.mybir as mybir
    from concourse.tile import TileContext

    f32 = mybir.dt.float32
    mult = mybir.AluOpType.mult
    add = mybir.AluOpType.add

    nc = bacc.Bacc(None)
    x = nc.declare_dram_parameter("x", [TOK, C], f32, isOutput=False)
    cab = nc.declare_dram_parameter("coef_ab", [1, 2 * C], f32, isOutput=False)
    w = nc.declare_dram_parameter("wblk", [P, C], f32, isOutput=False)
    ident = nc.declare_dram_parameter("ident", [P, P], f32, isOutput=False)
    y = nc.declare_dram_parameter("y", [TOK, C], f32, isOutput=True)

    # [TOK, C] viewed as [SS, P, S, C]: slab ss, partition p = token row
    # (ss*S*P + s*P + p). Each slab is a contiguous 4 MiB DRAM region;
    # per-partition chunks are 8 KiB contiguous.
    xv = x[:, :].rearrange("(ss s p) c -> ss p s c", p=P, s=S)
    yv = y[:, :].rearrange("(ss s p) c -> ss p s c", p=P, s=S)

    NSTR = SS * S  # 32 global stripes
    PREFETCH = 7  # stripes in flight

    def stripe_ap(view, k):
        return view[k // S][:, k % S, :]

    with TileContext(nc) as tc:
        with (
            tc.tile_pool(name="coef", bufs=1) as coefp,
            tc.tile_pool(name="xp", bufs=PREFETCH) as xp,
            tc.tile_pool(name="yp", bufs=6) as yp,
            tc.tile_pool(name="tp", bufs=2) as tp,
            tc.tile_pool(name="up", bufs=2) as up,
            tc.tile_pool(name="xtp", bufs=2) as xtp,
            tc.tile_pool(name="pst", bufs=2, space="PSUM") as pst,
            tc.tile_pool(name="psy", bufs=2, space="PSUM") as psy,
        ):
            a_sb = coefp.tile([P, C], f32)
            b_sb = coefp.tile([P, C], f32)
            w_sb = coefp.tile([P, C], f32)
            id_sb = coefp.tile([P, P], f32)
            ab_row = coefp.tile([1, 2 * C], f32)
            ones_row = coefp.tile([1, P], f32)
            # Small loads ride the ACT HWDGE ring so they overlap the first
            # x-stripe loads on the sync ring.
            nc.scalar.dma_start(ab_row[:], cab[:, :])
            nc.scalar.dma_start(w_sb[:], w[:, :])
            nc.scalar.dma_start(id_sb[:], ident[:, :])
            nc.vector.memset(ones_row[:], 1.0)
            # Broadcast the [1, C] coefficient rows to [P, C] on-device via
            # K=1 PE matmuls (ones_row.T @ ab_row chunk) — saves ~2 MB of
            # DMA on an otherwise DMA-bound kernel; PE is idle at start.
            for ci, dst in ((0, a_sb), (1, b_sb)):
                for n in range(C // 512):
                    psB = psy.tile([P, 512], f32)
                    nc.tensor.matmul(
                        psB[:],
                        ones_row[:],
                        ab_row[:, ci * C + n * 512 : ci * C + (n + 1) * 512],
                    )
                    nc.scalar.copy(dst[:, n * 512 : (n + 1) * 512], psB[:])
            b3 = b_sb[:].rearrange("p (g two) -> p g two", two=2)

            # Fully stripe-granular pipeline: 1 MiB loads/stores per
            # 128-token stripe, PREFETCH stripes in flight. Issue order on
            # the sync ring is load(k+PREFETCH) right after store(k), so
            # loads always prefetch instead of queueing behind a
            # wait-for-compute store.
            def load_stripe(k):
                xst = xp.tile([P, C], f32, tag="xs")
                nc.sync.dma_start(xst[:], stripe_ap(xv, k))
                return xst

            x_tiles = {k: load_stripe(k) for k in range(PREFETCH)}

            for k in range(NSTR):
                xs = x_tiles.pop(k)[:, :]
                ys_tile = yp.tile([P, C], f32, tag="ys")
                ys = ys_tile[:, :]
                if _is_pe_stripe(k):
                    # PE path: per 128x128 block, transpose then matmul.
                    for a in range(NBLK // 4):
                        psT = pst.tile([P, 512], f32)
                        for b in range(4):
                            j = 4 * a + b
                            nc.tensor.transpose(
                                psT[:, b * P : (b + 1) * P],
                                xs[:, j * P : (j + 1) * P],
                                id_sb[:],
                            )
                        xt = xtp.tile([P, 512], f32)
                        nc.scalar.copy(xt[:], psT[:])
                        psY = psy.tile([P, 512], f32)
                        for b in range(4):
                            j = 4 * a + b
                            nc.tensor.matmul(
                                psY[:, b * P : (b + 1) * P],
                                xt[:, b * P : (b + 1) * P],
                                w_sb[:, j * P : (j + 1) * P],
                            )
                        nc.scalar.copy(ys[:, a * 512 : (a + 1) * 512], psY[:])
                else:
                    # DVE path: three fp32 TT ops.
                    xs_sw = xs.rearrange("p (g two) -> p g two", two=2)[
                        :, :, ::-1
                    ]
                    t = tp.tile([P, C], f32)
                    u = up.tile([P, C], f32)
                    nc.vector.tensor_tensor(t[:], xs, a_sb[:], mult)
                    nc.vector.tensor_tensor(
                        u[:].rearrange("p (g two) -> p g two", two=2),
                        xs_sw,
                        b3,
                        mult,
                    )
                    nc.vector.tensor_tensor(ys, t[:], u[:], add)
                nc.sync.dma_start(stripe_ap(yv, k), ys)
                if k + PREFETCH < NSTR:
                    x_tiles[k + PREFETCH] = load_stripe(k + PREFETCH)

    nc.finalize()
    _NC_CACHE["nc"] = nc
    return nc


def run(x, W_pairs, phase, trace=False):
    """Run on 8 NeuronCores; returns (y_full, BassKernelResults)."""
    from concourse.bass_utils import run_bass_kernel_spmd

    x = np.ascontiguousarray(np.asarray(x, dtype=np.float32))
    assert x.shape == (B_FULL, T_FULL, C), x.shape
    A, Bc, W_dram = _prep_coeffs(W_pairs, phase)
    ab_row = np.ascontiguousarray(
        np.concatenate([A, Bc]).reshape(1, 2 * C)
    )
    ident = np.eye(P, dtype=np.float32)

    nc = _build_nc()
    in_maps = [
        {
            "x": x[core].reshape(TOK, C),
            "coef_ab": ab_row,
            "wblk": W_dram,
            "ident": ident,
        }
        for core in range(N_CORES)
    ]
    res = run_bass_kernel_spmd(nc, in_maps, list(range(N_CORES)), trace=trace)
    y = np.stack([res.results[i]["y"] for i in range(N_CORES)], axis=0)
    return y.reshape(B_FULL, T_FULL, C), res


def kernel(x, W_pairs, phase):
    y, _ = run(x, W_pairs, phase)
    return y


# revision 27
# speedup vs baseline: 1.1869x; 1.1869x over previous
"""Trainium2 Bass kernel for a per-channel-pair 2x2 unitary mixing layer.

Math (reference): for each channel pair g (C=2048 -> G=1024 pairs):
    M[g] = R(phase[g]) @ polar_project(W_pairs[g])        # 2x2
    y[..., 2g:2g+2] = M[g] @ x[..., 2g:2g+2]

Device formulation (token-major):
    A[2g] = M[g,0,0]; A[2g+1] = M[g,1,1]
    B[2g] = M[g,0,1]; B[2g+1] = M[g,1,0]
    y[c] = A[c] * x[c] + B[c] * x[partner(c)]
The tiny 2x2 SVD/compose prep runs on host (same formula as the
reference); the [B,T,C] transform runs on 8 NeuronCores, data parallel
over batch (x[b] per core), with coefficient tensors replicated.

Per core the work is split across engine pipelines so the kernel rides
the HBM roofline (~206 us for 64 MiB at the per-NC limit):
  - DVE path (6 of 8 slabs): three fp32 tensor_tensor ops per stripe
    (t = x*A; u = swap(x)*B; y = t + u). ~27.5 us/slab.
  - PE path (2 of 8 slabs): per 128x128 block, PE transpose to PSUM,
    ACT copy to SBUF, PE matmul against the host-built block-diagonal
    W chunk (y = x @ Wblk), ACT copy back. ~45 us/slab, runs on
    otherwise-idle engines.
GPSIMD is left idle: its SBUF port serializes against DVE 2-read ops
(exclusive shared-port lock, measured). All DMAs are HWDGE (nc.sync),
4 MiB per transfer.
"""

import sys

if "/opt/trn_rl_repo" not in sys.path:
    sys.path.insert(0, "/opt/trn_rl_repo")

import numpy as np

# Problem shape (hardcoded per harness contract)
B_FULL, T_FULL, C = 8, 4096, 2048
G = C // 2
N_CORES = 8
TOK = T_FULL  # tokens per core after batch sharding: 4096
P = 128  # SBUF partitions
S = 4  # token-stripes per slab
SS = TOK // (S * P)  # slabs per core: 8
NBLK = C // P  # 128-channel blocks per stripe: 16

# Global stripes processed on the PE (transpose+matmul) path; the rest go
# to the DVE path. ~1 of 3 balances PE (~12 us/stripe wall) against DVE
# (~6.9 us/stripe) under the ~162 us DMA floor.
def _is_pe_stripe(k):
    return k % 3 == 2

_NC_CACHE = {}


def _prep_coeffs(W_pairs: np.ndarray, phase: np.ndarray):
    """Host prep: M = R(phase) @ polar(W).

    Returns per-channel coefficient vectors A, B and the block-diagonal
    weight chunks W_dram [P, C] used by the PE path (W_dram[p, j*128+n]
    = Wblk_j[p, n], y_block = x_block @ Wblk_j).
    """
    W = np.asarray(W_pairs, dtype=np.float32)
    ph = np.asarray(phase, dtype=np.float32)
    # Same math as the reference: polar factor via SVD (U @ Vh), fp32.
    U, _, Vh = np.linalg.svd(W)
    Q = U @ Vh  # [G,2,2]
    c, s = np.cos(ph), np.sin(ph)
    R = np.stack([np.stack([c, -s], -1), np.stack([s, c], -1)], -2)  # [G,2,2]
    M = np.einsum("gij,gjk->gik", R, Q).astype(np.float32)  # [G,2,2]

    A = np.empty(C, dtype=np.float32)
    Bc = np.empty(C, dtype=np.float32)
    A[0::2] = M[:, 0, 0]
    A[1::2] = M[:, 1, 1]
    Bc[0::2] = M[:, 0, 1]
    Bc[1::2] = M[:, 1, 0]

    # Block-diagonal chunks: Wblk_j[2gl+jj, 2gl+ii] = M[j*64+gl, ii, jj]
    Wblk = np.zeros((NBLK, P, P), dtype=np.float32)
    gl = np.arange(P // 2)
    for j in range(NBLK):
        Mj = M[j * (P // 2) + gl]  # [64,2,2]
        for jj in range(2):
            for ii in range(2):
                Wblk[j, 2 * gl + jj, 2 * gl + ii] = Mj[:, ii, jj]
    W_dram = np.ascontiguousarray(Wblk.transpose(1, 0, 2).reshape(P, C))
    return A, Bc, W_dram


def _build_nc():
    """Build the single-core Bass program (SPMD across 8 cores)."""
    if "nc" in _NC_CACHE:
        return _NC_CACHE["nc"]

    import concourse.bacc as bacc
    import concourse.mybir as mybir
    from concourse.tile import TileContext

    f32 = mybir.dt.float32
    mult = mybir.AluOpType.mult
    add = mybir.AluOpType.add

    nc = bacc.Bacc(None)
    x = nc.declare_dram_parameter("x", [TOK, C], f32, isOutput=False)
    ca = nc.declare_dram_parameter("coef_a", [P, C], f32, isOutput=False)
    cb = nc.declare_dram_parameter("coef_b", [P, C], f32, isOutput=False)
    w = nc.declare_dram_parameter("wblk", [P, C], f32, isOutput=False)
    ident = nc.declare_dram_parameter("ident", [P, P], f32, isOutput=False)
    y = nc.declare_dram_parameter("y", [TOK, C], f32, isOutput=True)

    # [TOK, C] viewed as [SS, P, S, C]: slab ss, partition p = token row
    # (ss*S*P + s*P + p). Each slab is a contiguous 4 MiB DRAM region;
    # per-partition chunks are 8 KiB contiguous.
    xv = x[:, :].rearrange("(ss s p) c -> ss p s c", p=P, s=S)
    yv = y[:, :].rearrange("(ss s p) c -> ss p s c", p=P, s=S)

    NSTR = SS * S  # 32 global stripes
    PREFETCH = 8  # stripes in flight (2 slabs ahead)

    def stripe_ap(view, k):
        return view[k // S][:, k % S, :]

    with TileContext(nc) as tc:
        with (
            tc.tile_pool(name="coef", bufs=1) as coefp,
            tc.tile_pool(name="xp", bufs=PREFETCH) as xp,
            tc.tile_pool(name="yp", bufs=PREFETCH) as yp,
            tc.tile_pool(name="tp", bufs=2) as tp,
            tc.tile_pool(name="up", bufs=2) as up,
            tc.tile_pool(name="xtp", bufs=2) as xtp,
            tc.tile_pool(name="pst", bufs=2, space="PSUM") as pst,
            tc.tile_pool(name="psy", bufs=2, space="PSUM") as psy,
        ):
            a_sb = coefp.tile([P, C], f32)
            b_sb = coefp.tile([P, C], f32)
            w_sb = coefp.tile([P, C], f32)
            id_sb = coefp.tile([P, P], f32)
            # Coefficient loads ride the ACT HWDGE ring so they overlap the
            # first x-stripe loads on the sync ring.
            nc.scalar.dma_start(a_sb[:], ca[:, :])
            nc.scalar.dma_start(b_sb[:], cb[:, :])
            nc.scalar.dma_start(w_sb[:], w[:, :])
            nc.scalar.dma_start(id_sb[:], ident[:, :])
            b3 = b_sb[:].rearrange("p (g two) -> p g two", two=2)

            # Fully stripe-granular pipeline: 1 MiB loads/stores per
            # 128-token stripe, PREFETCH stripes in flight. Issue order on
            # the sync ring is load(k+PREFETCH) right after store(k), so
            # loads always prefetch instead of queueing behind a
            # wait-for-compute store.
            def load_stripe(k):
                xst = xp.tile([P, C], f32, tag="xs")
                nc.sync.dma_start(xst[:], stripe_ap(xv, k))
                return xst

            x_tiles = {k: load_stripe(k) for k in range(PREFETCH)}

            for k in range(NSTR):
                xs = x_tiles.pop(k)[:, :]
                ys_tile = yp.tile([P, C], f32, tag="ys")
                ys = ys_tile[:, :]
                if _is_pe_stripe(k):
                    # PE path: per 128x128 block, transpose then matmul.
                    for a in range(NBLK // 4):
                        psT = pst.tile([P, 512], f32)
                        for b in range(4):
                            j = 4 * a + b
                            nc.tensor.transpose(
                                psT[:, b * P : (b + 1) * P],
                                xs[:, j * P : (j + 1) * P],
                                id_sb[:],
                            )
                        xt = xtp.tile([P, 512], f32)
                        nc.scalar.copy(xt[:], psT[:])
                        psY = psy.tile([P, 512], f32)
                        for b in range(4):
                            j = 4 * a + b
                            nc.tensor.matmul(
                                psY[:, b * P : (b + 1) * P],
                                xt[:, b * P : (b + 1) * P],
                                w_sb[:, j * P : (j + 1) * P],
                            )
                        nc.scalar.copy(ys[:, a * 512 : (a + 1) * 512], psY[:])
                else:
                    # DVE path: three fp32 TT ops.
                    xs_sw = xs.rearrange("p (g two) -> p g two", two=2)[
                        :, :, ::-1
                    ]
                    t = tp.tile([P, C], f32)
                    u = up.tile([P, C], f32)
                    nc.vector.tensor_tensor(t[:], xs, a_sb[:], mult)
                    nc.vector.tensor_tensor(
                        u[:].rearrange("p (g two) -> p g two", two=2),
                        xs_sw,
                        b3,
                        mult,
                    )
                    nc.vector.tensor_tensor(ys, t[:], u[:], add)
                nc.sync.dma_start(stripe_ap(yv, k), ys)
                if k + PREFETCH < NSTR:
                    x_tiles[k + PREFETCH] = load_stripe(k + PREFETCH)

    nc.finalize()
    _NC_CACHE["nc"] = nc
    return nc


def run(x, W_pairs, phase, trace=False):
    """Run on 8 NeuronCores; returns (y_full, BassKernelResults)."""
    from concourse.bass_utils import run_bass_kernel_spmd

    x = np.ascontiguousarray(np.asarray(x, dtype=np.float32))
    assert x.shape == (B_FULL, T_FULL, C), x.shape
    A, Bc, W_dram = _prep_coeffs(W_pairs, phase)
    a_bc = np.ascontiguousarray(np.broadcast_to(A, (P, C)))
    b_bc = np.ascontiguousarray(np.broadcast_to(Bc, (P, C)))
    ident = np.eye(P, dtype=np.float32)

    nc = _build_nc()
    in_maps = [
        {
            "x": x[core].reshape(TOK, C),
            "coef_a": a_bc,
            "coef_b": b_bc,
            "wblk": W_dram,
            "ident": ident,
        }
        for core in range(N_CORES)
    ]
    res = run_bass_kernel_spmd(nc, in_maps, list(range(N_CORES)), trace=trace)
    y = np.stack([res.results[i]["y"] for i in range(N_CORES)], axis=0)
    return y.reshape(B_FULL, T_FULL, C), res


def kernel(x, W_pairs, phase):
    y, _ = run(x, W_pairs, phase)
    return y
